# revision 3
# baseline (speedup 1.0000x reference)
"""Distributed Trainium2 kernel for GQA attention (B=2, T=2048, D=2048, N=8
query heads, K=1 KV head, H=256) on 8 NeuronCores.

Sharding: 2 (batch) x 4 (head-pair) mesh. Core c handles batch c//4 and query
heads {2*(c%4), 2*(c%4)+1}. K/V projections are computed per batch group
(replicated across the 4 cores of a group). The per-head out-projection
partial sums are reduced with a chunked ReduceScatter over replica groups
[[0,1,2,3],[4,5,6,7]]; the host concatenates the row shards.

Device-side layout ("transposed attention"):
  xT [D, T] (host pre-transposed, bf16)
  qT/kT [H, T] = proj + rope (rope tables host-precomputed from positions;
                 the H^-0.5 scale is folded into wq on the host)
  logitsT [S-chunk(128), T-blk(512)] = kT-chunk^T @ qT   (PSUM f32)
  expT = exp(logitsT) via ScalarE, bf16; causal masking via 4 static
         diagonal 0/1 tiles; fully-masked chunks are skipped entirely.
  dsum [1, T-blk] += ones^T @ expT (PE), recip via DVE, broadcast across
         partitions with a K=1 ones outer-product matmul, folded into the
         encodedT PSUM->SBUF copy.
  encT [H, T] += v-chunk^T @ expT
  out [T-tile(128), D-blk(512)] = sum_h encT_h^T @ wo_h  -> bf16 -> RS.
"""

import numpy as np
import ml_dtypes

import concourse.bass as bass
import concourse.bacc as bacc
import concourse.mybir as mybir
import concourse.tile as tile
from concourse import bass_utils

BF = mybir.dt.bfloat16
F32 = mybir.dt.float32

B, T, D, N, KVH, H = 2, 2048, 2048, 8, 1, 256
MAX_WAVELENGTH = 10000
TBLK = 512   # T block (matmul moving free dim / PSUM bank)
SCH = 128    # S chunk (key chunk, PSUM partition dim)
TT = 128     # T tile (out-projection partition dim)
GROUPS = [[0, 1, 2, 3], [4, 5, 6, 7]]
N_CORES = 8


def build(causal=True, t=T, d=D):
    """Build the SPMD graph (identical on all 8 cores)."""
    n_tblk = t // TBLK
    n_dch = d // 128      # D chunks (contraction)
    n_dblk = d // TBLK    # D blocks (out free dim)
    n_sch = t // SCH      # S chunks
    n_tt = TBLK // TT     # T tiles per T block

    nc = bacc.Bacc("TRN2", target_bir_lowering=False, debug=False,
                   num_devices=N_CORES)

    xT_e = nc.dram_tensor("xT", [d, t], BF, kind="ExternalInput")
    wq_e = nc.dram_tensor("wq", [d, 2 * H], BF, kind="ExternalInput")
    wk_e = nc.dram_tensor("wk", [d, H], BF, kind="ExternalInput")
    wv_e = nc.dram_tensor("wv", [d, H], BF, kind="ExternalInput")
    wo_e = nc.dram_tensor("wo", [2 * H, d], BF, kind="ExternalInput")
    cos_e = nc.dram_tensor("cosT", [H // 2, t], F32, kind="ExternalInput")
    sin_e = nc.dram_tensor("sinT", [H // 2, t], F32, kind="ExternalInput")
    if causal:
        cm_e = nc.dram_tensor("cmask", [SCH, 4 * TBLK], BF, kind="ExternalInput")
    else:
        gm_e = nc.dram_tensor("gmask", [t, t], BF, kind="ExternalInput")
    out_e = nc.dram_tensor("out", [t // 4, d], BF, kind="ExternalOutput")

    with tile.TileContext(nc) as tc:
        # SBUF pool stack (LIFO release): persist, tmps, then W (released
        # mid-kernel, replaced by phaseB pool).
        poolP = tc.alloc_tile_pool(name="persist", bufs=1)
        poolT = tc.alloc_tile_pool(name="tmps", bufs=4)
        poolPS = tc.alloc_tile_pool(name="ps", bufs=1, space="PSUM")
        poolD = tc.alloc_tile_pool(name="dram", bufs=1, space="DRAM")
        poolW = tc.alloc_tile_pool(name="w", bufs=1)

        # ---- input loads -------------------------------------------------
        xts = [poolW.tile([128, t], BF, name=f"xt{i}") for i in range(n_dch)]
        wqs = [poolW.tile([128, 2 * H], BF, name=f"wqt{i}") for i in range(n_dch)]
        wks = [poolW.tile([128, H], BF, name=f"wkt{i}") for i in range(n_dch)]
        wvs = [poolW.tile([128, H], BF, name=f"wvt{i}") for i in range(n_dch)]
        wos = [poolP.tile([128, d], BF, name=f"wot{k}") for k in range(4)]
        cos_sb = poolP.tile([128, t], F32, name="cos_sb")
        sin_sb = poolP.tile([128, t], F32, name="sin_sb")
        for i in range(n_dch):
            nc.sync.dma_start(wqs[i][:], wq_e.ap()[128 * i:128 * (i + 1), :])
            nc.sync.dma_start(wks[i][:], wk_e.ap()[128 * i:128 * (i + 1), :])
            nc.sync.dma_start(wvs[i][:], wv_e.ap()[128 * i:128 * (i + 1), :])
        for k in range(4):
            nc.sync.dma_start(wos[k][:], wo_e.ap()[128 * k:128 * (k + 1), :])
        nc.sync.dma_start(cos_sb[:], cos_e.ap()[:, :])
        nc.sync.dma_start(sin_sb[:], sin_e.ap()[:, :])
        if causal:
            cm_sb = poolP.tile([SCH, 4 * TBLK], BF, name="cm_sb")
            nc.sync.dma_start(cm_sb[:], cm_e.ap()[:, :])
        for i in range(n_dch):
            nc.sync.dma_start(xts[i][:], xT_e.ap()[128 * i:128 * (i + 1), :])

        ones_col = poolP.tile([128, 1], BF, name="ones_col")
        nc.vector.memset(ones_col[:], 1.0)
        ones_row = poolP.tile([1, 128], F32, name="ones_row")
        nc.vector.memset(ones_row[:], 1.0)

        # ---- phase A: projections + rope --------------------------------
        v_sb = [poolP.tile([128, H], BF, name=f"v{j}") for j in range(n_sch)]
        # v[j] = xT[:, j-chunk]^T @ wv   -> [128 (S), H]
        for j in range(n_sch):
            ps_v = poolPS.tile([128, H], F32, name="ps_v",
                               tag="wo" if j % 2 == 0 else "aux", bufs=2)
            for di in range(n_dch):
                nc.tensor.matmul(ps_v[:], xts[di][:, j * SCH:(j + 1) * SCH],
                                 wvs[di][:], start=(di == 0),
                                 stop=(di == n_dch - 1))
            nc.vector.tensor_copy(v_sb[j][:], ps_v[:])

        def proj_rope(top_dst, bot_dst, w_tiles, col0, m):
            """Project+rope one T block: dst[:, m*TBLK:(m+1)*TBLK]."""
            sl = slice(m * TBLK, (m + 1) * TBLK)
            ps_top = poolPS.tile([128, TBLK], F32, name="ps_top", tag="qk", bufs=2)
            ps_bot = poolPS.tile([128, TBLK], F32, name="ps_bot", tag="enc", bufs=2)
            for di in range(n_dch):
                nc.tensor.matmul(ps_top[:], w_tiles[di][:, col0:col0 + 128],
                                 xts[di][:, sl], start=(di == 0),
                                 stop=(di == n_dch - 1))
            for di in range(n_dch):
                nc.tensor.matmul(ps_bot[:], w_tiles[di][:, col0 + 128:col0 + 256],
                                 xts[di][:, sl], start=(di == 0),
                                 stop=(di == n_dch - 1))
            c_sl, s_sl = cos_sb[:, sl], sin_sb[:, sl]
            t1 = poolT.tile([128, TBLK], F32, name="rt1", tag="tmp")
            t2 = poolT.tile([128, TBLK], F32, name="rt2", tag="tmp")
            nc.vector.tensor_mul(t1[:], ps_top[:], c_sl)
            nc.vector.tensor_mul(t2[:], ps_bot[:], s_sl)
            nc.vector.tensor_sub(top_dst[:, sl], t1[:], t2[:])
            t3 = poolT.tile([128, TBLK], F32, name="rt3", tag="tmp")
            t4 = poolT.tile([128, TBLK], F32, name="rt4", tag="tmp")
            nc.vector.tensor_mul(t3[:], ps_bot[:], c_sl)
            nc.vector.tensor_mul(t4[:], ps_top[:], s_sl)
            nc.vector.tensor_add(bot_dst[:, sl], t3[:], t4[:])

        ktop = poolP.tile([128, t], BF, name="ktop")
        kbot = poolP.tile([128, t], BF, name="kbot")
        for m in range(n_tblk):
            proj_rope(ktop, kbot, wks, 0, m)
        qtop = [poolP.tile([128, t], BF, name=f"qtop{h}") for h in range(2)]
        qbot = [poolP.tile([128, t], BF, name=f"qbot{h}") for h in range(2)]
        for h in range(2):
            for m in range(n_tblk):
                proj_rope(qtop[h], qbot[h], wqs, h * H, m)

        poolW.release()
        poolB = tc.alloc_tile_pool(name="phaseB", bufs=1)
        if not causal:
            poolG = tc.alloc_tile_pool(name="gmask", bufs=4)

        encT = [poolB.tile([128, t], BF, name=f"enc{k}") for k in range(4)]
        in_bounce = poolD.tile([t, d], BF, name="in_bounce")
        out_bounces = [poolD.tile([TBLK // 4, d], BF, name=f"out_b{m}")
                       for m in range(n_tblk)]

        # ---- phase B: attention + out-projection + chunked RS -----------
        for m in range(n_tblk):
            t_sl = slice(m * TBLK, (m + 1) * TBLK)
            n_chunks = 4 * (m + 1) if causal else n_sch
            for h in range(2):
                ps_e0 = poolPS.tile([128, TBLK], F32, name="ps_e0", tag="enc", bufs=2)
                ps_e1 = poolPS.tile([128, TBLK], F32, name="ps_e1", tag="enc", bufs=2)
                ps_ds = poolPS.tile([1, TBLK], F32, name="ps_ds", tag="aux", bufs=2)
                for j in range(n_chunks):
                    s_sl = slice(j * SCH, (j + 1) * SCH)
                    ps_l = poolPS.tile([128, TBLK], F32, name="ps_l", tag="qk", bufs=2)
                    nc.tensor.matmul(ps_l[:], ktop[:, s_sl], qtop[h][:, t_sl],
                                     start=True, stop=False)
                    nc.tensor.matmul(ps_l[:], kbot[:, s_sl], qbot[h][:, t_sl],
                                     start=False, stop=True)
                    ex = poolB.tile([128, TBLK], BF, name="ex", tag="ex", bufs=6)
                    nc.scalar.activation(ex[:], ps_l[:],
                                         mybir.ActivationFunctionType.Exp)
                    if causal:
                        if j >= 4 * m:  # diagonal chunk
                            i = j - 4 * m
                            nc.vector.tensor_mul(
                                ex[:], ex[:], cm_sb[:, i * TBLK:(i + 1) * TBLK])
                    else:
                        gm = poolG.tile([128, TBLK], BF, name="gm", tag="gm")
                        nc.sync.dma_start(gm[:], gm_e.ap()[s_sl, t_sl])
                        nc.vector.tensor_mul(ex[:], ex[:], gm[:])
                    last = j == n_chunks - 1
                    nc.tensor.matmul(ps_e0[:], v_sb[j][:, 0:128], ex[:],
                                     start=(j == 0), stop=last)
                    nc.tensor.matmul(ps_e1[:], v_sb[j][:, 128:256], ex[:],
                                     start=(j == 0), stop=last)
                    nc.tensor.matmul(ps_ds[:], ones_col[:], ex[:],
                                     start=(j == 0), stop=last)
                # softmax denominator -> per-partition broadcast reciprocal
                rrow = poolB.tile([1, TBLK], F32, name="rrow", tag="rrow", bufs=2)
                nc.vector.reciprocal(rrow[:], ps_ds[:])
                ps_rb = poolPS.tile([128, TBLK], F32, name="ps_rb", tag="aux", bufs=2)
                nc.tensor.matmul(ps_rb[:], ones_row[:], rrow[:],
                                 start=True, stop=True)
                rbc = poolB.tile([128, TBLK], F32, name="rbc", tag="rbc", bufs=2)
                nc.vector.tensor_copy(rbc[:], ps_rb[:])
                nc.vector.tensor_mul(encT[2 * h][:, t_sl], ps_e0[:], rbc[:])
                nc.vector.tensor_mul(encT[2 * h + 1][:, t_sl], ps_e1[:], rbc[:])

            # out-projection for this T block (both heads)
            for tt in range(n_tt):
                r_sl = slice(m * TBLK + tt * TT, m * TBLK + (tt + 1) * TT)
                for db in range(n_dblk):
                    d_sl = slice(db * TBLK, (db + 1) * TBLK)
                    ps_o = poolPS.tile([128, TBLK], F32, name="ps_o", tag="wo", bufs=2)
                    for k in range(4):
                        nc.tensor.matmul(ps_o[:], encT[k][:, r_sl],
                                         wos[k][:, d_sl], start=(k == 0),
                                         stop=(k == 3))
                    ostg = poolB.tile([128, TBLK], BF, name="ostg", tag="ostg", bufs=4)
                    nc.vector.tensor_copy(ostg[:], ps_o[:])
                    nc.sync.dma_start(in_bounce[r_sl, d_sl], ostg[:])
            # ReduceScatter this T block across the 4-core batch group
            nc.gpsimd.collective_compute(
                "ReduceScatter", mybir.AluOpType.add, replica_groups=GROUPS,
                ins=[in_bounce[t_sl, :].opt()], outs=[out_bounces[m].opt()])
            nc.sync.dma_start(
                out_e.ap()[m * (TBLK // 4):(m + 1) * (TBLK // 4), :],
                out_bounces[m][:])

        if not causal:
            poolG.release()
        poolB.release()
        poolD.release()
        poolPS.release()
        poolT.release()
        poolP.release()

    nc.compile()
    return nc


_NC_CACHE = {}


def _get_nc(causal, t=T, d=D):
    key = (causal, t, d)
    if key not in _NC_CACHE:
        _NC_CACHE[key] = build(causal, t, d)
    return _NC_CACHE[key]


def _rope_tables(pos):
    """pos [T] f32 -> cosT, sinT [H/2, T] f32."""
    half = H // 2
    freq_exp = (2.0 / H) * np.arange(half, dtype=np.float32)
    timescale = (MAX_WAVELENGTH ** freq_exp).astype(np.float32)
    radians = pos[None, :].astype(np.float32) / timescale[:, None]
    return np.cos(radians).astype(np.float32), np.sin(radians).astype(np.float32)


def _causal_tiles():
    """4 diagonal 0/1 tiles [SCH, TBLK]: tile i -> 1{ds + 128*i <= dt}."""
    ds = np.arange(SCH)[:, None]
    dt = np.arange(TBLK)[None, :]
    tiles = [(dt >= ds + SCH * i).astype(np.float32) for i in range(4)]
    return np.concatenate(tiles, axis=1).astype(ml_dtypes.bfloat16)


def _prep_in_maps(x, positions, attn_mask, wq, wkv, wo, causal):
    bf = ml_dtypes.bfloat16
    scale = np.float32(H) ** np.float32(-0.5)
    wq_s = (np.asarray(wq, np.float32) * scale)
    wk = np.asarray(wkv[0, 0], np.float32).astype(bf)
    wv = np.asarray(wkv[1, 0], np.float32).astype(bf)
    cm = _causal_tiles() if causal else None

    in_maps = []
    for c in range(N_CORES):
        b, r = divmod(c, 4)
        h0, h1 = 2 * r, 2 * r + 1
        xT = np.ascontiguousarray(np.asarray(x[b], np.float32).T).astype(bf)
        wq_c = np.ascontiguousarray(
            np.concatenate([wq_s[h0], wq_s[h1]], axis=1)).astype(bf)
        wo_c = np.ascontiguousarray(
            np.concatenate([np.asarray(wo[h0], np.float32),
                            np.asarray(wo[h1], np.float32)], axis=0)).astype(bf)
        cosT, sinT = _rope_tables(np.asarray(positions[b], np.float32))
        m = {"xT": xT, "wq": wq_c, "wk": wk, "wv": wv, "wo": wo_c,
             "cosT": cosT, "sinT": sinT}
        if causal:
            m["cmask"] = cm
        else:
            m["gmask"] = np.ascontiguousarray(
                np.asarray(attn_mask[b, 0], np.float32).T).astype(bf)
        in_maps.append(m)
    return in_maps


def kernel(x, positions, attn_mask, wq, wkv, wo):
    x = np.asarray(x)
    positions = np.asarray(positions)
    attn_mask = np.asarray(attn_mask)
    wq, wkv, wo = np.asarray(wq), np.asarray(wkv), np.asarray(wo)

    tril = np.tril(np.ones((T, T), bool))
    causal = all(np.array_equal(attn_mask[b, 0], tril) for b in range(B))

    nc = _get_nc(causal)
    in_maps = _prep_in_maps(x, positions, attn_mask, wq, wkv, wo, causal)
    res = bass_utils.run_bass_kernel_spmd(nc, in_maps,
                                          core_ids=list(range(N_CORES)))

    out = np.empty((B, T, D), np.float32)
    n_tblk = T // TBLK
    rows = TBLK // 4  # 128 rows per core per RS chunk
    for c in range(N_CORES):
        b, r = divmod(c, 4)
        shard = np.asarray(res.results[c]["out"], dtype=np.float32)
        for m in range(n_tblk):
            t0 = m * TBLK + r * rows
            out[b, t0:t0 + rows, :] = shard[m * rows:(m + 1) * rows, :]
    return out


# revision 4
# speedup vs baseline: 1.0934x; 1.0934x over previous
"""Distributed Trainium2 kernel for GQA attention (B=2, T=2048, D=2048, N=8
query heads, K=1 KV head, H=256) on 8 NeuronCores.

Sharding: 2 (batch) x 4 (head-pair) mesh. Core c handles batch c//4 and query
heads {2*(c%4), 2*(c%4)+1}. K/V projections are computed per batch group
(replicated across the 4 cores of a group). The per-head out-projection
partial sums are reduced with a chunked ReduceScatter over replica groups
[[0,1,2,3],[4,5,6,7]]; the host concatenates the row shards.

Device-side layout ("transposed attention"):
  xT [D, T] (host pre-transposed, bf16)
  qT/kT [H, T] = proj + rope (rope tables host-precomputed from positions;
                 the H^-0.5 scale is folded into wq on the host)
  logitsT [S-chunk(128), T-blk(512)] = kT-chunk^T @ qT   (PSUM f32)
  expT = exp(logitsT) via ScalarE, bf16; causal masking via 4 static
         diagonal 0/1 tiles; fully-masked chunks are skipped entirely.
  dsum [1, T-blk] += ones^T @ expT (PE), recip via DVE, broadcast across
         partitions with a K=1 ones outer-product matmul, folded into the
         encodedT normalization multiply.
  encT [H, T] += v-chunk^T @ expT
  out [T-tile(128), D-blk(512)] = sum_h encT_h^T @ wo_h  -> bf16 -> RS.

Scheduling notes: engine queues are in-order, so the key loops are
software-pipelined at emission time (logits/exp of chunk j+1 are emitted
before the PV matmuls of chunk j; rope runs one block behind the
projection matmuls). xT is DMA'd in column blocks interleaved with the
weights so the first projections start ~10us in instead of waiting for
all 16 MB of input.
"""

import numpy as np
import ml_dtypes

import concourse.bass as bass
import concourse.bacc as bacc
import concourse.mybir as mybir
import concourse.tile as tile
from concourse import bass_utils

BF = mybir.dt.bfloat16
F32 = mybir.dt.float32

B, T, D, N, KVH, H = 2, 2048, 2048, 8, 1, 256
MAX_WAVELENGTH = 10000
TBLK = 512    # T block (matmul moving free dim / PSUM bank)
SCH = 128     # S chunk (key chunk, PSUM partition dim)
TT = 128      # T tile (out-projection partition dim)
RS_ROWS = 256  # rows per ReduceScatter chunk
GROUPS = [[0, 1, 2, 3], [4, 5, 6, 7]]
N_CORES = 8


def build(causal=True, t=T, d=D):
    """Build the SPMD graph (identical on all 8 cores)."""
    n_tblk = t // TBLK
    n_dch = d // 128
    n_dblk = d // TBLK
    n_sch = t // SCH
    n_tt = TBLK // TT
    n_rs = t // RS_ROWS           # RS chunks
    rs_out = RS_ROWS // 4         # rows per core per RS chunk

    nc = bacc.Bacc("TRN2", target_bir_lowering=False, debug=False,
                   num_devices=N_CORES)

    xT_e = nc.dram_tensor("xT", [d, t], BF, kind="ExternalInput")
    wq_e = nc.dram_tensor("wq", [d, 2 * H], BF, kind="ExternalInput")
    wk_e = nc.dram_tensor("wk", [d, H], BF, kind="ExternalInput")
    wv_e = nc.dram_tensor("wv", [d, H], BF, kind="ExternalInput")
    wo_e = nc.dram_tensor("wo", [2 * H, d], BF, kind="ExternalInput")
    cos_e = nc.dram_tensor("cosT", [H // 2, t], F32, kind="ExternalInput")
    sin_e = nc.dram_tensor("sinT", [H // 2, t], F32, kind="ExternalInput")
    if causal:
        cm_e = nc.dram_tensor("cmask", [SCH, 4 * TBLK], BF, kind="ExternalInput")
    else:
        gm_e = nc.dram_tensor("gmask", [t, t], BF, kind="ExternalInput")
    out_e = nc.dram_tensor("out", [t // 4, d], BF, kind="ExternalOutput")

    with tile.TileContext(nc) as tc:
        poolP = tc.alloc_tile_pool(name="persist", bufs=1)
        poolT = tc.alloc_tile_pool(name="tmps", bufs=4)
        poolPS = tc.alloc_tile_pool(name="ps", bufs=1, space="PSUM")
        poolD = tc.alloc_tile_pool(name="dram", bufs=1, space="DRAM")
        poolW = tc.alloc_tile_pool(name="w", bufs=1)

        # ---- input loads: wv + xT block 0 first so v-proj starts early ---
        xts = [poolW.tile([128, t], BF, name=f"xt{i}") for i in range(n_dch)]
        wqs = [poolW.tile([128, 2 * H], BF, name=f"wqt{i}") for i in range(n_dch)]
        wks = [poolW.tile([128, H], BF, name=f"wkt{i}") for i in range(n_dch)]
        wvs = [poolW.tile([128, H], BF, name=f"wvt{i}") for i in range(n_dch)]
        wos = [poolP.tile([128, d], BF, name=f"wot{k}") for k in range(4)]
        cos_sb = poolP.tile([128, t], F32, name="cos_sb")
        sin_sb = poolP.tile([128, t], F32, name="sin_sb")

        def load_x_block(blk):
            sl = slice(blk * TBLK, (blk + 1) * TBLK)
            for i in range(n_dch):
                nc.sync.dma_start(xts[i][:, sl],
                                  xT_e.ap()[128 * i:128 * (i + 1), sl])

        for i in range(n_dch):
            nc.sync.dma_start(wvs[i][:], wv_e.ap()[128 * i:128 * (i + 1), :])
        load_x_block(0)
        for i in range(n_dch):
            nc.sync.dma_start(wks[i][:], wk_e.ap()[128 * i:128 * (i + 1), :])
        nc.sync.dma_start(cos_sb[:], cos_e.ap()[:, :])
        nc.sync.dma_start(sin_sb[:], sin_e.ap()[:, :])
        if n_tblk > 1:
            load_x_block(1)
        for i in range(n_dch):
            nc.sync.dma_start(wqs[i][:], wq_e.ap()[128 * i:128 * (i + 1), :])
        for blk in range(2, n_tblk):
            load_x_block(blk)
        for k in range(4):
            nc.sync.dma_start(wos[k][:], wo_e.ap()[128 * k:128 * (k + 1), :])
        if causal:
            cm_sb = poolP.tile([SCH, 4 * TBLK], BF, name="cm_sb")
            nc.sync.dma_start(cm_sb[:], cm_e.ap()[:, :])

        ones_col = poolP.tile([128, 1], BF, name="ones_col")
        nc.vector.memset(ones_col[:], 1.0)
        ones_row = poolP.tile([1, 128], F32, name="ones_row")
        nc.vector.memset(ones_row[:], 1.0)

        # ---- phase A: projections + rope --------------------------------
        v_sb = [poolP.tile([128, H], BF, name=f"v{j}") for j in range(n_sch)]
        for j in range(n_sch):
            ps_v = poolPS.tile([128, H], F32, name="ps_v",
                               tag="wo" if j % 2 == 0 else "aux", bufs=2)
            for di in range(n_dch):
                nc.tensor.matmul(ps_v[:], xts[di][:, j * SCH:(j + 1) * SCH],
                                 wvs[di][:], start=(di == 0),
                                 stop=(di == n_dch - 1))
            nc.vector.tensor_copy(v_sb[j][:], ps_v[:])

        ktop = poolP.tile([128, t], BF, name="ktop")
        kbot = poolP.tile([128, t], BF, name="kbot")
        qtop = [poolP.tile([128, t], BF, name=f"qtop{h}") for h in range(2)]
        qbot = [poolP.tile([128, t], BF, name=f"qbot{h}") for h in range(2)]

        def emit_proj(w_tiles, col0, m):
            sl = slice(m * TBLK, (m + 1) * TBLK)
            ps_top = poolPS.tile([128, TBLK], F32, name="ps_top", tag="qk", bufs=2)
            ps_bot = poolPS.tile([128, TBLK], F32, name="ps_bot", tag="enc", bufs=2)
            for di in range(n_dch):
                nc.tensor.matmul(ps_top[:], w_tiles[di][:, col0:col0 + 128],
                                 xts[di][:, sl], start=(di == 0),
                                 stop=(di == n_dch - 1))
            for di in range(n_dch):
                nc.tensor.matmul(ps_bot[:], w_tiles[di][:, col0 + 128:col0 + 256],
                                 xts[di][:, sl], start=(di == 0),
                                 stop=(di == n_dch - 1))
            return ps_top, ps_bot

        def emit_rope(job):
            top_dst, bot_dst, m, ps_top, ps_bot = job
            sl = slice(m * TBLK, (m + 1) * TBLK)
            c_sl, s_sl = cos_sb[:, sl], sin_sb[:, sl]
            t1 = poolT.tile([128, TBLK], F32, name="rt1", tag="tmp")
            t2 = poolT.tile([128, TBLK], F32, name="rt2", tag="tmp")
            nc.vector.tensor_mul(t1[:], ps_top[:], c_sl)
            nc.vector.tensor_mul(t2[:], ps_bot[:], s_sl)
            nc.vector.tensor_sub(top_dst[:, sl], t1[:], t2[:])
            t3 = poolT.tile([128, TBLK], F32, name="rt3", tag="tmp")
            t4 = poolT.tile([128, TBLK], F32, name="rt4", tag="tmp")
            nc.vector.tensor_mul(t3[:], ps_bot[:], c_sl)
            nc.vector.tensor_mul(t4[:], ps_top[:], s_sl)
            nc.vector.tensor_add(bot_dst[:, sl], t3[:], t4[:])

        # rope runs one projection block behind so the PE never waits on DVE
        proj_seq = [(ktop, kbot, wks, 0, m) for m in range(n_tblk)]
        for h in range(2):
            proj_seq += [(qtop[h], qbot[h], wqs, h * H, m) for m in range(n_tblk)]
        pending = None
        for (top_dst, bot_dst, w_tiles, col0, m) in proj_seq:
            ps_top, ps_bot = emit_proj(w_tiles, col0, m)
            if pending is not None:
                emit_rope(pending)
            pending = (top_dst, bot_dst, m, ps_top, ps_bot)
        emit_rope(pending)

        poolW.release()
        poolB = tc.alloc_tile_pool(name="phaseB", bufs=1)
        if not causal:
            poolG = tc.alloc_tile_pool(name="gmask", bufs=4)

        encT = [poolB.tile([128, t], BF, name=f"enc{k}") for k in range(4)]
        in_bounce = poolD.tile([t, d], BF, name="in_bounce")
        out_bounces = [poolD.tile([rs_out, d], BF, name=f"out_b{i}")
                       for i in range(n_rs)]

        # ---- phase B: attention + out-projection + chunked RS -----------
        for m in range(n_tblk):
            t_sl = slice(m * TBLK, (m + 1) * TBLK)
            n_chunks = 4 * (m + 1) if causal else n_sch

            for h in range(2):
                ps_e0 = poolPS.tile([128, TBLK], F32, name="ps_e0", tag="enc", bufs=2)
                ps_e1 = poolPS.tile([128, TBLK], F32, name="ps_e1", tag="enc", bufs=2)
                ps_ds = poolPS.tile([1, TBLK], F32, name="ps_ds", tag="aux", bufs=2)

                def emit_logits_exp(j):
                    s_sl = slice(j * SCH, (j + 1) * SCH)
                    ps_l = poolPS.tile([128, TBLK], F32, name="ps_l", tag="qk", bufs=2)
                    nc.tensor.matmul(ps_l[:], ktop[:, s_sl], qtop[h][:, t_sl],
                                     start=True, stop=False)
                    nc.tensor.matmul(ps_l[:], kbot[:, s_sl], qbot[h][:, t_sl],
                                     start=False, stop=True)
                    ex = poolB.tile([128, TBLK], BF, name="ex", tag="ex", bufs=6)
                    nc.scalar.activation(ex[:], ps_l[:],
                                         mybir.ActivationFunctionType.Exp)
                    if causal:
                        if j >= 4 * m:
                            i = j - 4 * m
                            nc.vector.tensor_mul(
                                ex[:], ex[:], cm_sb[:, i * TBLK:(i + 1) * TBLK])
                    else:
                        gm = poolG.tile([128, TBLK], BF, name="gm", tag="gm")
                        nc.sync.dma_start(gm[:], gm_e.ap()[s_sl, t_sl])
                        nc.vector.tensor_mul(ex[:], ex[:], gm[:])
                    return ex

                # software pipeline: logits/exp of j+1 issue before PV of j
                ex_next = emit_logits_exp(0)
                for j in range(n_chunks):
                    ex = ex_next
                    if j + 1 < n_chunks:
                        ex_next = emit_logits_exp(j + 1)
                    last = j == n_chunks - 1
                    nc.tensor.matmul(ps_e0[:], v_sb[j][:, 0:128], ex[:],
                                     start=(j == 0), stop=last)
                    nc.tensor.matmul(ps_e1[:], v_sb[j][:, 128:256], ex[:],
                                     start=(j == 0), stop=last)
                    nc.tensor.matmul(ps_ds[:], ones_col[:], ex[:],
                                     start=(j == 0), stop=last)

                # free the enc PSUM banks immediately; normalize later
                ef0 = poolB.tile([128, TBLK], F32, name="ef0", tag="ef", bufs=4)
                ef1 = poolB.tile([128, TBLK], F32, name="ef1", tag="ef", bufs=4)
                nc.vector.tensor_copy(ef0[:], ps_e0[:])
                nc.vector.tensor_copy(ef1[:], ps_e1[:])
                rrow = poolB.tile([1, TBLK], F32, name="rrow", tag="rrow", bufs=2)
                nc.vector.reciprocal(rrow[:], ps_ds[:])
                ps_rb = poolPS.tile([128, TBLK], F32, name="ps_rb", tag="aux", bufs=2)
                nc.tensor.matmul(ps_rb[:], ones_row[:], rrow[:],
                                 start=True, stop=True)
                nc.vector.tensor_mul(encT[2 * h][:, t_sl], ef0[:], ps_rb[:])
                nc.vector.tensor_mul(encT[2 * h + 1][:, t_sl], ef1[:], ps_rb[:])

            # out-projection for this T block; RS every RS_ROWS rows
            for tt in range(n_tt):
                r_sl = slice(m * TBLK + tt * TT, m * TBLK + (tt + 1) * TT)
                for db in range(n_dblk):
                    d_sl = slice(db * TBLK, (db + 1) * TBLK)
                    ps_o = poolPS.tile([128, TBLK], F32, name="ps_o", tag="wo", bufs=2)
                    for k in range(4):
                        nc.tensor.matmul(ps_o[:], encT[k][:, r_sl],
                                         wos[k][:, d_sl], start=(k == 0),
                                         stop=(k == 3))
                    ostg = poolB.tile([128, TBLK], BF, name="ostg", tag="ostg", bufs=4)
                    nc.vector.tensor_copy(ostg[:], ps_o[:])
                    nc.sync.dma_start(in_bounce[r_sl, d_sl], ostg[:])
                done_rows = m * TBLK + (tt + 1) * TT
                if done_rows % RS_ROWS == 0:
                    i = done_rows // RS_ROWS - 1
                    nc.gpsimd.collective_compute(
                        "ReduceScatter", mybir.AluOpType.add,
                        replica_groups=GROUPS,
                        ins=[in_bounce[i * RS_ROWS:(i + 1) * RS_ROWS, :].opt()],
                        outs=[out_bounces[i].opt()])
                    nc.sync.dma_start(
                        out_e.ap()[i * rs_out:(i + 1) * rs_out, :],
                        out_bounces[i][:])

        if not causal:
            poolG.release()
        poolB.release()
        poolD.release()
        poolPS.release()
        poolT.release()
        poolP.release()

    nc.compile()
    return nc


_NC_CACHE = {}


def _get_nc(causal, t=T, d=D):
    key = (causal, t, d)
    if key not in _NC_CACHE:
        _NC_CACHE[key] = build(causal, t, d)
    return _NC_CACHE[key]


def _rope_tables(pos):
    """pos [T] f32 -> cosT, sinT [H/2, T] f32."""
    half = H // 2
    freq_exp = (2.0 / H) * np.arange(half, dtype=np.float32)
    timescale = (MAX_WAVELENGTH ** freq_exp).astype(np.float32)
    radians = pos[None, :].astype(np.float32) / timescale[:, None]
    return np.cos(radians).astype(np.float32), np.sin(radians).astype(np.float32)


def _causal_tiles():
    """4 diagonal 0/1 tiles [SCH, TBLK]: tile i -> 1{ds + 128*i <= dt}."""
    ds = np.arange(SCH)[:, None]
    dt = np.arange(TBLK)[None, :]
    tiles = [(dt >= ds + SCH * i).astype(np.float32) for i in range(4)]
    return np.concatenate(tiles, axis=1).astype(ml_dtypes.bfloat16)


def _prep_in_maps(x, positions, attn_mask, wq, wkv, wo, causal):
    bf = ml_dtypes.bfloat16
    scale = np.float32(H) ** np.float32(-0.5)
    wq_s = (np.asarray(wq, np.float32) * scale)
    wk = np.asarray(wkv[0, 0], np.float32).astype(bf)
    wv = np.asarray(wkv[1, 0], np.float32).astype(bf)
    cm = _causal_tiles() if causal else None

    in_maps = []
    for c in range(N_CORES):
        b, r = divmod(c, 4)
        h0, h1 = 2 * r, 2 * r + 1
        xT = np.ascontiguousarray(np.asarray(x[b], np.float32).T).astype(bf)
        wq_c = np.ascontiguousarray(
            np.concatenate([wq_s[h0], wq_s[h1]], axis=1)).astype(bf)
        wo_c = np.ascontiguousarray(
            np.concatenate([np.asarray(wo[h0], np.float32),
                            np.asarray(wo[h1], np.float32)], axis=0)).astype(bf)
        cosT, sinT = _rope_tables(np.asarray(positions[b], np.float32))
        m = {"xT": xT, "wq": wq_c, "wk": wk, "wv": wv, "wo": wo_c,
             "cosT": cosT, "sinT": sinT}
        if causal:
            m["cmask"] = cm
        else:
            m["gmask"] = np.ascontiguousarray(
                np.asarray(attn_mask[b, 0], np.float32).T).astype(bf)
        in_maps.append(m)
    return in_maps


def kernel(x, positions, attn_mask, wq, wkv, wo):
    x = np.asarray(x)
    positions = np.asarray(positions)
    attn_mask = np.asarray(attn_mask)
    wq, wkv, wo = np.asarray(wq), np.asarray(wkv), np.asarray(wo)

    tril = np.tril(np.ones((T, T), bool))
    causal = all(np.array_equal(attn_mask[b, 0], tril) for b in range(B))

    nc = _get_nc(causal)
    in_maps = _prep_in_maps(x, positions, attn_mask, wq, wkv, wo, causal)
    res = bass_utils.run_bass_kernel_spmd(nc, in_maps,
                                          core_ids=list(range(N_CORES)))

    out = np.empty((B, T, D), np.float32)
    n_rs = T // RS_ROWS
    rs_out = RS_ROWS // 4
    for c in range(N_CORES):
        b, r = divmod(c, 4)
        shard = np.asarray(res.results[c]["out"], dtype=np.float32)
        for i in range(n_rs):
            t0 = i * RS_ROWS + r * rs_out
            out[b, t0:t0 + rs_out, :] = shard[i * rs_out:(i + 1) * rs_out, :]
    return out
